# revision 8
# baseline (speedup 1.0000x reference)
"""Causal self-attention Trainium2 kernel (Bass/Tile), data-parallel over batch.

Problem: y = CausalSelfAttention(x) with B=8, T=1024, C=1024, H=16 heads.
Sharding: batch-parallel -- each of the 8 NeuronCores handles one batch
element end-to-end; weights are replicated. No collectives needed.

Per-core pipeline (everything stays on-chip between stages):
  1. load x [T,C], transpose via PE -> xT [C,T]
  2. V = x @ Wv + bv, kept untransposed [T,C] with an extra ones column per
     head (gives the softmax denominator for free in step 4)
     Q,K computed transposed: qkT [2C, T], bias via per-partition
     tensor_scalar_add
  3. per head: scoresT [Tk,Tq] = k @ q^T (2 heads packed in the PE array via
     row tiling), causal tiles only; exp via ACT (scale folded in); 0/1 mask
     multiply on diagonal-crossing tiles (DVE)
  4. yT_aug [65, Tq] = [v | 1]^T-matmul over attT -> row 64 is l_i = sum_j att
     normalize yT by 1/l (DMA round-trip partition broadcast), then
  5. out = y @ Wp + bp, written back [T, C]

Matmuls run as float32r (full-rate fp32 PE mode, 1 cycle/row at N=512).
"""

from contextlib import ExitStack

import numpy as np

T = 1024
C = 1024
H = 16
HD = 64
NPAIR = 8  # head pairs; pair p = heads (2p, 2p+1)
P = 128
QW = 512  # q-column width per scores tile
SCALE = 1.0 / np.sqrt(HD)

_CACHE = {}


def _build():
    import concourse.bass as bass
    import concourse.tile as tile
    from concourse import bacc, mybir
    from concourse.masks import make_identity

    f32 = mybir.dt.float32
    f32r = mybir.dt.float32r
    AF = mybir.ActivationFunctionType

    nc = bacc.Bacc("TRN2", target_bir_lowering=False, debug=False)

    x_d = nc.dram_tensor("x", [T, C], f32, kind="ExternalInput")
    wa_d = nc.dram_tensor("w_attn", [C, 3 * C], f32r, kind="ExternalInput")
    ba_d = nc.dram_tensor("b_attn", [3 * C], f32, kind="ExternalInput")
    wp_d = nc.dram_tensor("w_proj", [C, C], f32r, kind="ExternalInput")
    bp_d = nc.dram_tensor("b_proj", [C], f32, kind="ExternalInput")
    out_d = nc.dram_tensor("out", [T, C], f32, kind="ExternalOutput")

    def r(ap):
        return ap.bitcast(f32r)

    def bcast_ap(src, n):
        # broadcast a [1, F] (or 1D [F]) AP across n partitions via step-0
        free = list(src.ap)
        if len(free) > 1:
            free = free[1:]
        return bass.AP(tensor=src.tensor, offset=src.offset, ap=[[0, n]] + free)

    with tile.TileContext(nc) as tc, ExitStack() as outer:
        ep = outer.enter_context
        consts = ep(tc.tile_pool(name="consts", bufs=1))
        yT_pool = ep(tc.tile_pool(name="yT", bufs=1))
        dram_pool = ep(tc.tile_pool(name="dram", bufs=1, space="DRAM"))
        ps_mm = ep(tc.tile_pool(name="ps_mm", bufs=2, space="PSUM"))

        # stacks closed at phase boundaries to release SBUF/PSUM
        mid = ExitStack()  # xT + attention-phase pools
        xT_pool = mid.enter_context(tc.tile_pool(name="xT", bufs=1))
        v_pool = mid.enter_context(tc.tile_pool(name="v", bufs=1))

        # ---------------- constants ----------------
        ident = consts.tile([P, P], f32)
        make_identity(nc, ident)

        # causal keep-masks for diagonal-crossing tiles:
        # masks[:, oi, i] row j -> 1.0 if j + 128*oi <= i else 0.0
        masks = consts.tile([P, 4, QW], f32)
        nc.gpsimd.memset(masks, 1.0)
        for oi in range(4):
            nc.gpsimd.affine_select(
                out=masks[:, oi, :],
                in_=masks[:, oi, :],
                compare_op=mybir.AluOpType.is_ge,
                fill=0.0,
                base=-(128 * oi),
                pattern=[[1, QW]],
                channel_multiplier=-1,
            )

        # qk bias, per-partition layout: b_qk[p, j] = b_attn[128j + p]
        b_qk = consts.tile([P, 16], f32)
        ba_cols = ba_d.ap().rearrange("(j p) -> p j", p=P)
        nc.sync.dma_start(out=b_qk, in_=ba_cols[:, 0:16])

        # v bias and proj bias broadcast to all 128 partitions
        b_v = consts.tile([P, C], f32)
        nc.gpsimd.dma_start(out=b_v, in_=bcast_ap(ba_d.ap()[2 * C : 3 * C], P))
        b_p = consts.tile([P, C], f32)
        nc.gpsimd.dma_start(out=b_p, in_=bcast_ap(bp_d.ap(), P))

        ones16 = consts.tile([P, H, 1], f32)
        nc.vector.memset(ones16, 1.0)

        lr_dram = dram_pool.tile([H, T], f32)

        # ---------------- load x + transpose (PE) ----------------
        xT = [xT_pool.tile([P, T], f32r, tag=f"xT{cc}", name=f"xT{cc}")
              for cc in range(8)]
        with tc.tile_pool(name="xload", bufs=3) as xload, \
                tc.tile_pool(name="ps_tp", bufs=3, space="PSUM") as ps_tp:
            for t in range(8):
                xt = xload.tile([P, C], f32, tag="xld")
                nc.sync.dma_start(out=xt, in_=x_d.ap()[t * P : (t + 1) * P, :])
                for cg in range(2):
                    tp = ps_tp.tile([P, 512], f32, tag="tp")
                    for k in range(4):
                        cc = 4 * cg + k
                        nc.tensor.transpose(
                            out=tp[:, k * P : (k + 1) * P],
                            in_=xt[:, cc * P : (cc + 1) * P],
                            identity=ident,
                        )
                    for k in range(4):
                        cc = 4 * cg + k
                        nc.vector.tensor_copy(
                            out=xT[cc][:, t * P : (t + 1) * P],
                            in_=tp[:, k * P : (k + 1) * P],
                        )

        # ---------------- V (untransposed, with ones col) ----------------
        v_sb = [v_pool.tile([P, H, HD + 1], f32r, tag=f"v{t}", name=f"v{t}")
                for t in range(8)]
        with tc.tile_pool(name="wv", bufs=1) as wv_pool:
            w_v = wv_pool.tile([P, 8, C], f32r)
            nc.sync.dma_start(
                out=w_v,
                in_=wa_d.ap()[:, 2 * C : 3 * C].rearrange(
                    "(cc p) n -> p cc n", p=P
                ),
            )
            for t in range(8):
                vv = v_sb[t]
                for nh in range(2):
                    ps = ps_mm.tile([P, 512], f32, tag="mmps")
                    for cc in range(8):
                        nc.tensor.matmul(
                            ps,
                            lhsT=(xT[cc][:, t * P : (t + 1) * P]),
                            rhs=(w_v[:, cc, nh * 512 : (nh + 1) * 512]),
                            start=(cc == 0),
                            stop=(cc == 7),
                        )
                    nc.vector.tensor_add(
                        out=vv[:, nh * 8 : (nh + 1) * 8, 0:HD],
                        in0=ps.rearrange("p (h e) -> p h e", e=HD),
                        in1=b_v[:, nh * 512 : (nh + 1) * 512].rearrange(
                            "p (h e) -> p h e", e=HD
                        ),
                    )
                nc.vector.tensor_copy(out=vv[:, :, HD : HD + 1], in_=ones16)

        # ---------------- per pair: QK chunks then attention ----------------
        wcol_pool = mid.enter_context(tc.tile_pool(name="wcol", bufs=3))
        qk_pool = mid.enter_context(tc.tile_pool(name="qk", bufs=2))
        att_pool = mid.enter_context(tc.tile_pool(name="att", bufs=9))
        lsb_pool = mid.enter_context(tc.tile_pool(name="lsb", bufs=2))
        lbc_pool = mid.enter_context(tc.tile_pool(name="lbc", bufs=2))
        ps_sc = mid.enter_context(tc.tile_pool(name="ps_sc", bufs=4, space="PSUM"))
        ps_av = mid.enter_context(tc.tile_pool(name="ps_av", bufs=2, space="PSUM"))
        yT = [yT_pool.tile([P, T], f32r, tag=f"yT{p}", name=f"yT{p}")
              for p in range(NPAIR)]
        for p in range(NPAIR):
            # q chunk (j = p) and k chunk (j = 8 + p), transposed layout
            qk_tiles = {}
            for which, j in (("q", p), ("k", 8 + p)):
                wcol = wcol_pool.tile([P, 8, P], f32r, tag="wcol")
                nc.sync.dma_start(
                    out=wcol,
                    in_=wa_d.ap()[:, j * P : (j + 1) * P].rearrange(
                        "(cc p) n -> p cc n", p=P
                    ),
                )
                dst = qk_pool.tile([P, T], f32r, tag=f"{which}t", name=f"{which}t")
                for qc in range(2):
                    ps = ps_mm.tile([P, 512], f32, tag="mmps")
                    for cc in range(8):
                        nc.tensor.matmul(
                            ps,
                            lhsT=(wcol[:, cc, :]),
                            rhs=(xT[cc][:, qc * 512 : (qc + 1) * 512]),
                            start=(cc == 0),
                            stop=(cc == 7),
                        )
                    nc.vector.tensor_scalar_add(
                        out=dst[:, qc * 512 : (qc + 1) * 512],
                        in0=ps,
                        scalar1=b_qk[:, j : j + 1],
                    )
                qk_tiles[which] = dst
            qt, kt = qk_tiles["q"], qk_tiles["k"]

            l_sb = [
                lsb_pool.tile([1, T], f32, tag=f"lsb{i}", name=f"lsb{i}")
                for i in range(2)
            ]
            for h_loc in range(2):
                base = h_loc * HD
                h = 2 * p + h_loc
                for qc in range(2):
                    kc_max = 4 * (qc + 1)
                    ats = []
                    for kc in range(kc_max):
                        sps = ps_sc.tile([P, QW], f32, tag="sps")
                        nc.tensor.matmul(
                            sps,
                            lhsT=(kt[base : base + HD, kc * P : (kc + 1) * P]),
                            rhs=(qt[base : base + HD, qc * QW : (qc + 1) * QW]),
                            start=True,
                            stop=True,
                        )
                        at = att_pool.tile([P, QW], f32r, tag="att")
                        nc.scalar.activation(
                            out=at, in_=sps, func=AF.Exp, scale=SCALE
                        )
                        o = kc * P - qc * QW
                        if o >= 0:
                            nc.vector.tensor_mul(
                                out=at, in0=at, in1=masks[:, o // P, :]
                            )
                        ats.append(at)
                    avps = ps_av.tile([HD + 1, QW], f32, tag="avps")
                    for kc in range(kc_max):
                        nc.tensor.matmul(
                            avps,
                            lhsT=(v_sb[kc][:, h, 0 : HD + 1]),
                            rhs=(ats[kc]),
                            start=(kc == 0),
                            stop=(kc == kc_max - 1),
                        )
                    nc.scalar.activation(
                        out=yT[p][base : base + HD, qc * QW : (qc + 1) * QW],
                        in_=avps[0:HD, :],
                        func=AF.Copy,
                        scale=1.0,
                    )
                    nc.vector.tensor_copy(
                        out=l_sb[h_loc][0:1, qc * QW : (qc + 1) * QW],
                        in_=avps[HD : HD + 1, :],
                    )

            # normalize yT[p] by 1/l via DMA round-trip broadcast
            for h_loc in range(2):
                nc.vector.reciprocal(out=l_sb[h_loc], in_=l_sb[h_loc])
                nc.sync.dma_start(
                    out=lr_dram[2 * p + h_loc : 2 * p + h_loc + 1, :],
                    in_=l_sb[h_loc],
                )
            lbc = lbc_pool.tile([P, T], f32, tag="lbc")
            for h_loc in range(2):
                src = lr_dram[2 * p + h_loc : 2 * p + h_loc + 1, :]
                nc.gpsimd.dma_start(
                    out=lbc[h_loc * HD : (h_loc + 1) * HD, :],
                    in_=bcast_ap(src, HD),
                )
            nc.vector.tensor_mul(out=yT[p], in0=yT[p], in1=lbc)

        mid.close()  # free attention-phase SBUF/PSUM

        # ---------------- projection ----------------
        with tc.tile_pool(name="wp", bufs=1) as wp_pool, \
                tc.tile_pool(name="osb", bufs=3) as osb_pool:
            w_p = wp_pool.tile([P, 8, C], f32r)
            nc.sync.dma_start(
                out=w_p, in_=wp_d.ap().rearrange("(cc p) n -> p cc n", p=P)
            )
            for t in range(8):
                osb = osb_pool.tile([P, C], f32, tag="osb")
                for nh in range(2):
                    ps = ps_mm.tile([P, 512], f32, tag="mmps")
                    for cc in range(8):
                        nc.tensor.matmul(
                            ps,
                            lhsT=(yT[cc][:, t * P : (t + 1) * P]),
                            rhs=(w_p[:, cc, nh * 512 : (nh + 1) * 512]),
                            start=(cc == 0),
                            stop=(cc == 7),
                        )
                    nc.vector.tensor_add(
                        out=osb[:, nh * 512 : (nh + 1) * 512],
                        in0=ps,
                        in1=b_p[:, nh * 512 : (nh + 1) * 512],
                    )
                nc.sync.dma_start(
                    out=out_d.ap()[t * P : (t + 1) * P, :], in_=osb
                )

    nc.compile()
    return nc


def _get_nc():
    if "nc" not in _CACHE:
        _CACHE["nc"] = _build()
    return _CACHE["nc"]


def kernel(x, w_attn, b_attn, w_proj, b_proj):
    from concourse.bass_utils import run_bass_kernel_spmd

    x = np.ascontiguousarray(np.asarray(x, dtype=np.float32))
    w_attn = np.ascontiguousarray(np.asarray(w_attn, dtype=np.float32))
    b_attn = np.ascontiguousarray(np.asarray(b_attn, dtype=np.float32))
    w_proj = np.ascontiguousarray(np.asarray(w_proj, dtype=np.float32))
    b_proj = np.ascontiguousarray(np.asarray(b_proj, dtype=np.float32))

    B = x.shape[0]
    assert B == 8, f"expected batch 8, got {B}"

    nc = _get_nc()
    in_maps = [
        {
            "x": x[b],
            "w_attn": w_attn,
            "b_attn": b_attn,
            "w_proj": w_proj,
            "b_proj": b_proj,
        }
        for b in range(B)
    ]
    res = run_bass_kernel_spmd(nc, in_maps, core_ids=list(range(B)))
    return np.stack([res.results[b]["out"] for b in range(B)], axis=0)
